# revision 24
# baseline (speedup 1.0000x reference)
"""Trainium2 Bass kernel for nn_BatchSplitFF (expert-choice MoE routing + FFN).

Strategy (data-parallel over batch, 1 batch per NeuronCore, 8 cores):
  - fp32 logits on PE in [es, tok] orientation (routing must match the fp32
    reference argmax; bf16/fp32r logits flip routing decisions), N=512
    moving chunks; routing reductions run per-chunk, overlapped with the
    next chunk's matmuls.
  - dispatch: SWDGE dma_gather with SBUF source (sbuf_tokens_per_rank mode)
    reads selected token rows from the SBUF-resident bf16 copy of x --
    no HBM traffic for the 16.8MB of gathered rows.
  - expert FFN in bf16 on PE, es-pairs processed together:
      up-proj: col-tiled pair (es0 -> array cols 0-63, es1 -> cols 64-127)
      sharing one PSUM [128,128] tile; single ACT relu+bias per pair.
      down-proj: row-tiled pair (f of es0 on array rows 0-63, es1 on rows
      64-127) with f2 moving, disjoint PSUM outputs.
  - all DRAM tensors are laid out host-side so every DMA is contiguous
    per partition (16KB+ descriptors; strided descriptors measured 3-10x
    slower).
  - un-permute: y rows written densely to DRAM (bf16) as [g, a, e, d],
    routing table (tids) exported; host does the final scatter-add combine
    in fp32 (8.4M adds = 0.04% of FLOPs).
"""

import numpy as np
import ml_dtypes

import concourse.bass as bass
import concourse.mybir as mybir
import concourse.tile as tile
from concourse import bacc
from concourse.bass_utils import run_bass_kernel_spmd

bf16 = ml_dtypes.bfloat16
f32 = mybir.dt.float32
bfl = mybir.dt.bfloat16
i16 = mybir.dt.int16
i32 = mybir.dt.int32

DM, NE, ES, ESZ = 1024, 16, 4, 64
NES = NE * ES            # 64 (e,s) expert pairs
SEQ = 2048
G = SEQ // NE            # 128 groups per core
KT = DM // 128           # 8 contraction tiles
NCORES = 8
EG = 8                   # es-pairs per group-iteration
NCALLS = NES // EG       # 8 group-iterations
GIDX = 512               # idxs per dma_gather call (HW-validated limit)
TC = 4                   # token chunks for logits (512 tokens each)
GC = G // TC             # groups per token chunk (32)

_CACHE = {}


def _build_program():
    nc = bacc.Bacc("TRN2", target_bir_lowering=False, debug=False,
                   enable_asserts=False, num_devices=1)

    # [p, tt, k, 512]: xT chunk tt contiguous 16KB per partition
    xTc = nc.dram_tensor("xTc", [128, TC * KT * 512], f32,
                         kind="ExternalInput").ap()
    # token t at partition t%128, free stripe t//128 (gather source)
    xbf = nc.dram_tensor("xbf", [128, 16 * DM], bfl, kind="ExternalInput").ap()
    # [p, k, 64]: controller, contiguous per partition
    c2c = nc.dram_tensor("c2c", [128, KT * NES], f32, kind="ExternalInput").ap()
    # [p, a, k, 512]: f1 slab a contiguous 8KB per partition
    f1s = nc.dram_tensor("f1s", [128, NCALLS * KT * 512], bfl,
                         kind="ExternalInput").ap()
    # [e01*64+f, pair, d]: f2 pair layout (slab of 4 pairs contiguous)
    f2p = nc.dram_tensor("f2p", [128, NES // 2, DM], bfl,
                         kind="ExternalInput").ap()
    biasP = nc.dram_tensor("biasP", [128, NES // 2], f32,
                           kind="ExternalInput").ap()
    tie512 = nc.dram_tensor("tie512", [NES, 512], f32, kind="ExternalInput").ap()
    tokid = nc.dram_tensor("tokid", [NES, SEQ], f32, kind="ExternalInput").ap()
    ident = nc.dram_tensor("ident", [NES, NES], f32, kind="ExternalInput").ap()
    # [g, a, e, d]: y written contiguously 4KB per partition per pair
    stage3 = nc.dram_tensor("stage3", [128, NCALLS * EG * DM], bfl,
                            kind="ExternalOutput").ap()
    tids = nc.dram_tensor("tids", [NES, G], i32, kind="ExternalOutput").ap()

    with tile.TileContext(nc) as tc:
        with (
            tc.tile_pool(name="consts", bufs=1) as consts,
            tc.tile_pool(name="route", bufs=1) as route,
            tc.tile_pool(name="idxp", bufs=1) as idxp,
            tc.tile_pool(name="wp", bufs=2) as wp,
        ):
            # ---- constants into SBUF (scalar HWDGE ring; sync ring is
            # reserved for the xT stream so logits start immediately) ----
            c_sb = consts.tile([128, KT, NES], f32)
            nc.scalar.dma_start(out=c_sb[:],
                                in_=c2c.rearrange("p (k e) -> p k e", k=KT))
            biasP_sb = consts.tile([128, NES // 2], f32)
            nc.scalar.dma_start(out=biasP_sb[:], in_=biasP)
            tie_sb = consts.tile([NES, 512], f32)
            nc.scalar.dma_start(out=tie_sb[:], in_=tie512)
            tokid_sb = consts.tile([NES, SEQ], f32)
            nc.scalar.dma_start(out=tokid_sb[:], in_=tokid)
            ident_sb = consts.tile([NES, NES], f32)
            nc.scalar.dma_start(out=ident_sb[:], in_=ident)

            logits_sb = route.tile([NES, SEQ], f32)
            xbf_sb = consts.tile([128, 16 * DM], bfl)

            gmax = route.tile([NES, G], f32)
            iseq = route.tile([NES, SEQ], f32)
            tid_f = route.tile([NES, G], f32)

            xTc_r = xTc.rearrange("p (c k t) -> p c k t", c=TC, k=KT)
            f1s_r = f1s.rearrange("p (a k t) -> p a k t", a=NCALLS, k=KT)
            stage3_r = stage3.rearrange("p (a e d) -> p a e d", a=NCALLS, e=EG)

            # ---- phase B: fp32 logits + per-chunk routing ----
            with (
                tc.tile_pool(name="xtp", bufs=2) as xtp,
                tc.tile_pool(name="psB", bufs=2, space="PSUM") as psB,
            ):
                # gather source on the scalar ring, overlapping the xT stream
                nc.scalar.dma_start(out=xbf_sb[:], in_=xbf)
                for tt in range(TC):
                    xt_t = xtp.tile([128, KT, 512], f32)
                    nc.sync.dma_start(out=xt_t[:], in_=xTc_r[:, tt])
                    psum_l = psB.tile([NES, 512], f32, space="PSUM")
                    for k in range(KT):
                        nc.tensor.matmul(psum_l[:], c_sb[:, k, :], xt_t[:, k, :],
                                         start=(k == 0), stop=(k == KT - 1))
                    lg = logits_sb[:, tt * 512:(tt + 1) * 512]
                    nc.vector.tensor_tensor(out=lg, in0=psum_l[:], in1=tie_sb[:],
                                            op=mybir.AluOpType.add)
                    # routing for this chunk's 32 groups
                    gsl = slice(tt * GC, (tt + 1) * GC)
                    nc.vector.tensor_reduce(
                        out=gmax[:, gsl],
                        in_=lg.rearrange("e (g t) -> e g t", t=NE),
                        axis=mybir.AxisListType.X, op=mybir.AluOpType.max)
                    nc.vector.tensor_tensor(
                        out=iseq[:, tt * 512:(tt + 1) * 512].rearrange(
                            "e (g t) -> e g t", t=NE),
                        in0=lg.rearrange("e (g t) -> e g t", t=NE),
                        in1=gmax[:, gsl].unsqueeze(2).to_broadcast([NES, GC, NE]),
                        op=mybir.AluOpType.is_equal)
                    # tsel computed in place over iseq
                    nc.vector.tensor_tensor(
                        out=iseq[:, tt * 512:(tt + 1) * 512],
                        in0=iseq[:, tt * 512:(tt + 1) * 512],
                        in1=tokid_sb[:, tt * 512:(tt + 1) * 512],
                        op=mybir.AluOpType.mult)
                    nc.vector.tensor_reduce(
                        out=tid_f[:, gsl],
                        in_=iseq[:, tt * 512:(tt + 1) * 512].rearrange(
                            "e (g t) -> e g t", t=NE),
                        axis=mybir.AxisListType.X, op=mybir.AluOpType.max)

                # export routing table for the host-side combine
                tid_i32 = route.tile([NES, G], i32)
                nc.vector.tensor_copy(out=tid_i32[:], in_=tid_f[:])
                nc.scalar.dma_start(out=tids, in_=tid_i32[:])

                # gather idx tiles: transpose tid into [16, gh, es] psum layout
                psum_idx = psB.tile([16, 8, NES], f32, space="PSUM", tag="psidx")
                for gh in range(8):
                    nc.tensor.transpose(
                        out=psum_idx[:, gh, :],
                        in_=tid_f[:, gh * 16:(gh + 1) * 16],
                        identity=ident_sb[:])
                # idx_mega [128, 16 calls x 32 cols]; call h covers 4 es:
                # col j = e*8+gh, value = tid(es=4h+e, g=gh*16+p)
                idx_mega = idxp.tile([128, 16 * (GIDX // 16)], i16)
                for h in range(16):
                    nc.vector.tensor_copy(
                        out=idx_mega[0:16, h * 32:(h + 1) * 32].rearrange(
                            "p (e g) -> p e g", g=8),
                        in_=psum_idx[:, :, h * 4:(h + 1) * 4].transpose([0, 2, 1]))
                # replicate idx rows to all 128 partitions (Q7 channel reads)
                nc.sync.dma_start(out=idx_mega[16:32, :], in_=idx_mega[0:16, :])
                nc.sync.dma_start(out=idx_mega[32:64, :], in_=idx_mega[0:32, :])
                nc.sync.dma_start(out=idx_mega[64:128, :], in_=idx_mega[0:64, :])

            # ---- phase D: per es-group FFN, es processed in col/row-tiled
            # pairs ----
            with (
                tc.tile_pool(name="sp", bufs=3) as sp,
                tc.tile_pool(name="yp", bufs=2) as yp,
                tc.tile_pool(name="hp", bufs=3) as hp,
                tc.tile_pool(name="psH", bufs=2, space="PSUM") as psH,
                tc.tile_pool(name="psY", bufs=3, space="PSUM") as psY,
            ):
                for a in range(NCALLS):
                    f1_sb = wp.tile([128, KT, EG * ESZ], bfl, tag="f1")
                    nc.sync.dma_start(out=f1_sb[:], in_=f1s_r[:, a])
                    f2p_sb = wp.tile([128, EG // 2, DM], bfl, tag="f2")
                    nc.sync.dma_start(
                        out=f2p_sb[:],
                        in_=f2p[:, a * (EG // 2):(a + 1) * (EG // 2), :])

                    selT_halves = []
                    for half in range(2):
                        selTh = sp.tile([128, KT, GIDX], bfl, tag=f"selT{half}",
                                        name=f"selT_{a}_{half}")
                        nc.gpsimd.dma_gather(
                            out_ap=selTh[:],
                            in_ap=xbf_sb[:],
                            idxs_ap=idx_mega[:, (2 * a + half) * 32:
                                             (2 * a + half + 1) * 32],
                            num_idxs=GIDX, num_idxs_reg=GIDX, elem_size=DM,
                            transpose=True,
                            sbuf_tokens_per_rank=128,
                            sbuf_free_dim_per_rank=2 * DM,
                            sbuf_free_dim_pad_per_rank=0,
                            sbuf_byte_offset=0)
                        selT_halves.append(selTh)

                    y_sb = yp.tile([128, EG, DM], bfl)
                    for pl in range(EG // 2):          # es pair (8a+2pl, +1)
                        s0, s1 = 2 * pl, 2 * pl + 1    # slots within a
                        psum_h = psH.tile([128, G], f32, space="PSUM")
                        # up-proj: col-tiled pair, es0 -> array cols 0-63,
                        # es1 -> cols 64-127
                        for k in range(KT):
                            st = (k == 0)
                            sp_ = (k == KT - 1)
                            sel0 = selT_halves[s0 // 4]
                            sel1 = selT_halves[s1 // 4]
                            nc.tensor.matmul(
                                psum_h[0:64, :],
                                f1_sb[:, k, s0 * ESZ:(s0 + 1) * ESZ],
                                sel0[:, k, (s0 % 4) * G:(s0 % 4 + 1) * G],
                                start=st, stop=sp_, tile_position=(0, 0))
                            nc.tensor.matmul(
                                psum_h[64:128, :],
                                f1_sb[:, k, s1 * ESZ:(s1 + 1) * ESZ],
                                sel1[:, k, (s1 % 4) * G:(s1 % 4 + 1) * G],
                                start=st, stop=sp_, tile_position=(0, 64))
                        h_pair = hp.tile([128, G], bfl)
                        nc.scalar.activation(
                            out=h_pair[:], in_=psum_h[:],
                            func=mybir.ActivationFunctionType.Relu,
                            bias=biasP_sb[:, a * 4 + pl:a * 4 + pl + 1],
                            scale=1.0)
                        # down-proj: row-tiled pair, f of es0 on array rows
                        # 0-63, es1 on rows 64-127; disjoint PSUM outputs.
                        for n in range(2):
                            psum_t = psY.tile([128, 2, 512], f32, space="PSUM")
                            nc.tensor.matmul(
                                psum_t[:, 0, :],
                                h_pair[0:64, :],
                                f2p_sb[0:64, pl, n * 512:(n + 1) * 512],
                                start=True, stop=True)
                            nc.tensor.matmul(
                                psum_t[:, 1, :],
                                h_pair[64:128, :],
                                f2p_sb[64:128, pl, n * 512:(n + 1) * 512],
                                start=True, stop=True)
                            if n == 0:
                                nc.vector.tensor_copy(
                                    out=y_sb[:, s0:s1 + 1, 0:512],
                                    in_=psum_t[:])
                            else:
                                nc.scalar.copy(
                                    out=y_sb[:, s0:s1 + 1, 512:1024],
                                    in_=psum_t[:])

                    # dense write: [g, a, e, d] contiguous per partition
                    nc.scalar.dma_start(out=stage3_r[:, a], in_=y_sb[:])

    nc.compile()
    return nc


def _host_prep(x, controller, f1, f2, bias):
    """Returns (shared_map, per_core_maps)."""
    x = np.asarray(x, dtype=np.float32)
    c2c = np.ascontiguousarray(
        np.asarray(controller, np.float32).reshape(KT, 128, NES)
        .transpose(1, 0, 2).reshape(128, KT * NES))
    f1s = np.ascontiguousarray(
        np.asarray(f1, np.float32).reshape(DM, NES * ESZ)
        .reshape(KT, 128, NCALLS, 512).transpose(1, 2, 0, 3)
        .reshape(128, NCALLS * KT * 512)).astype(bf16)
    f2_ = np.asarray(f2, np.float32).reshape(NES, ESZ, DM)
    # pair layout: partition p = e01*64 + f, col = pair index
    f2p = np.ascontiguousarray(
        f2_.reshape(NES // 2, 2, ESZ, DM).transpose(1, 2, 0, 3)
        .reshape(128, NES // 2, DM)).astype(bf16)
    biasP = np.ascontiguousarray(
        np.asarray(bias, np.float32).reshape(NES // 2, 2, ESZ)
        .transpose(1, 2, 0).reshape(128, NES // 2))
    tie = np.linspace(0.0, 1e-6, NE, dtype=np.float32)
    tie512 = np.broadcast_to(np.tile(tie, 512 // NE), (NES, 512)).copy()
    tokid = np.broadcast_to(np.arange(SEQ, dtype=np.float32), (NES, SEQ)).copy()
    ident = np.eye(NES, dtype=np.float32)
    shared = dict(c2c=c2c, f1s=f1s, f2p=f2p, biasP=biasP, tie512=tie512,
                  tokid=tokid, ident=ident)
    per_core = []
    for b in range(NCORES):
        xb = x[b]
        xT = xb.T  # [DM, SEQ]
        xTc = np.ascontiguousarray(
            xT.reshape(KT, 128, TC, 512).transpose(1, 2, 0, 3)
            .reshape(128, TC * KT * 512))
        per_core.append(dict(
            xTc=xTc,
            xbf=np.ascontiguousarray(
                xb.astype(bf16).reshape(16, 128, DM).swapaxes(0, 1)
                .reshape(128, 16 * DM)),
        ))
    return shared, per_core


def _run(inputs, trace=False, tmpdir=None, trace_cores=None):
    if "nc" not in _CACHE:
        _CACHE["nc"] = _build_program()
    nc = _CACHE["nc"]
    shared, per_core = _host_prep(
        inputs["x"], inputs["controller"], inputs["f1"], inputs["f2"],
        inputs["bias"])
    in_maps = [dict(shared, **pc) for pc in per_core]
    res = run_bass_kernel_spmd(
        nc, in_maps, core_ids=list(range(NCORES)), trace=trace, tmpdir=tmpdir,
        trace_cores=trace_cores)
    out = np.zeros((NCORES, SEQ, DM), dtype=np.float32)
    for b in range(NCORES):
        # stage3 [g, a, e, d] -> (a, e, g) = es-major contribution rows
        st = np.asarray(res.results[b]["stage3"]).astype(np.float32)
        st = st.reshape(128, NCALLS, EG, DM).transpose(1, 2, 0, 3)
        tid = np.asarray(res.results[b]["tids"]).reshape(-1)  # [es*G] token ids
        np.add.at(out[b], tid, st.reshape(NES * G, DM))
    return out, res


def kernel(**inputs) -> np.ndarray:
    out, _ = _run(inputs)
    return out


# revision 25
# speedup vs baseline: 1.1184x; 1.1184x over previous
"""Trainium2 Bass kernel for nn_BatchSplitFF (expert-choice MoE routing + FFN).

Strategy (data-parallel over batch, 1 batch per NeuronCore, 8 cores):
  - fp32 logits on PE in [es, tok] orientation (routing must match the fp32
    reference argmax; bf16/fp32r logits flip routing decisions), N=512
    moving chunks; routing reductions run per-chunk, overlapped with the
    next chunk's matmuls.
  - dispatch: SWDGE dma_gather with SBUF source (sbuf_tokens_per_rank mode)
    reads selected token rows from the SBUF-resident bf16 copy of x --
    no HBM traffic for the 16.8MB of gathered rows.
  - expert FFN in bf16 on PE, es-pairs processed together:
      up-proj: col-tiled pair (es0 -> array cols 0-63, es1 -> cols 64-127)
      sharing one PSUM [128,128] tile; single ACT relu+bias per pair.
      down-proj: row-tiled pair (f of es0 on array rows 0-63, es1 on rows
      64-127) with f2 moving, disjoint PSUM outputs.
  - all DRAM tensors are laid out host-side so every DMA is contiguous
    per partition (16KB+ descriptors; strided descriptors measured 3-10x
    slower).
  - un-permute: y rows written densely to DRAM (bf16) as [g, a, e, d],
    routing table (tids) exported; host does the final scatter-add combine
    in fp32 (8.4M adds = 0.04% of FLOPs).
"""

import numpy as np
import ml_dtypes

import concourse.bass as bass
import concourse.mybir as mybir
import concourse.tile as tile
from concourse import bacc
from concourse.bass_utils import run_bass_kernel_spmd

bf16 = ml_dtypes.bfloat16
f32 = mybir.dt.float32
bfl = mybir.dt.bfloat16
i16 = mybir.dt.int16
i32 = mybir.dt.int32

DM, NE, ES, ESZ = 1024, 16, 4, 64
NES = NE * ES            # 64 (e,s) expert pairs
SEQ = 2048
G = SEQ // NE            # 128 groups per core
KT = DM // 128           # 8 contraction tiles
NCORES = 8
EG = 8                   # es-pairs per group-iteration
NCALLS = NES // EG       # 8 group-iterations
GIDX = 512               # idxs per dma_gather call (HW-validated limit)
TC = 4                   # token chunks for logits (512 tokens each)
GC = G // TC             # groups per token chunk (32)

_CACHE = {}


def _build_program():
    nc = bacc.Bacc("TRN2", target_bir_lowering=False, debug=False,
                   enable_asserts=False, num_devices=1)

    # [p, tt, k, 512]: xT chunk tt contiguous 16KB per partition
    xTc = nc.dram_tensor("xTc", [128, TC * KT * 512], f32,
                         kind="ExternalInput").ap()
    # token t at partition t%128, free stripe t//128 (gather source)
    xbf = nc.dram_tensor("xbf", [128, 16 * DM], bfl, kind="ExternalInput").ap()
    # [p, k, 64]: controller, contiguous per partition
    c2c = nc.dram_tensor("c2c", [128, KT * NES], f32, kind="ExternalInput").ap()
    # [p, a, k, 512]: f1 slab a contiguous 8KB per partition
    f1s = nc.dram_tensor("f1s", [128, NCALLS * KT * 512], bfl,
                         kind="ExternalInput").ap()
    # [e01*64+f, pair, d]: f2 pair layout (slab of 4 pairs contiguous)
    f2p = nc.dram_tensor("f2p", [128, NES // 2, DM], bfl,
                         kind="ExternalInput").ap()
    biasP = nc.dram_tensor("biasP", [128, NES // 2], f32,
                           kind="ExternalInput").ap()
    tie512 = nc.dram_tensor("tie512", [NES, 512], f32, kind="ExternalInput").ap()
    tokid = nc.dram_tensor("tokid", [NES, SEQ], f32, kind="ExternalInput").ap()
    ident = nc.dram_tensor("ident", [NES, NES], f32, kind="ExternalInput").ap()
    # [g, a, e, d]: y written contiguously 4KB per partition per pair
    stage3 = nc.dram_tensor("stage3", [128, NCALLS * EG * DM], bfl,
                            kind="ExternalOutput").ap()
    tids = nc.dram_tensor("tids", [NES, G], i32, kind="ExternalOutput").ap()

    with tile.TileContext(nc) as tc:
        with (
            tc.tile_pool(name="consts", bufs=1) as consts,
            tc.tile_pool(name="route", bufs=1) as route,
            tc.tile_pool(name="idxp", bufs=1) as idxp,
            tc.tile_pool(name="wp", bufs=2) as wp,
        ):
            # ---- constants into SBUF ----
            c_sb = consts.tile([128, KT, NES], f32)
            nc.sync.dma_start(out=c_sb[:],
                              in_=c2c.rearrange("p (k e) -> p k e", k=KT))
            biasP_sb = consts.tile([128, NES // 2], f32)
            nc.sync.dma_start(out=biasP_sb[:], in_=biasP)
            tie_sb = consts.tile([NES, 512], f32)
            nc.sync.dma_start(out=tie_sb[:], in_=tie512)
            tokid_sb = consts.tile([NES, SEQ], f32)
            nc.sync.dma_start(out=tokid_sb[:], in_=tokid)
            ident_sb = consts.tile([NES, NES], f32)
            nc.sync.dma_start(out=ident_sb[:], in_=ident)

            logits_sb = route.tile([NES, SEQ], f32)
            xbf_sb = consts.tile([128, 16 * DM], bfl)

            gmax = route.tile([NES, G], f32)
            iseq = route.tile([NES, SEQ], f32)
            tid_f = route.tile([NES, G], f32)

            xTc_r = xTc.rearrange("p (c k t) -> p c k t", c=TC, k=KT)
            f1s_r = f1s.rearrange("p (a k t) -> p a k t", a=NCALLS, k=KT)
            stage3_r = stage3.rearrange("p (a e d) -> p a e d", a=NCALLS, e=EG)

            # ---- phase B: fp32 logits + per-chunk routing ----
            with (
                tc.tile_pool(name="xtp", bufs=2) as xtp,
                tc.tile_pool(name="psB", bufs=2, space="PSUM") as psB,
            ):
                for tt in range(TC):
                    xt_t = xtp.tile([128, KT, 512], f32)
                    nc.sync.dma_start(out=xt_t[:], in_=xTc_r[:, tt])
                    if tt == 1:
                        # gather source; needed once routing completes
                        nc.sync.dma_start(out=xbf_sb[:], in_=xbf)
                    psum_l = psB.tile([NES, 512], f32, space="PSUM")
                    for k in range(KT):
                        nc.tensor.matmul(psum_l[:], c_sb[:, k, :], xt_t[:, k, :],
                                         start=(k == 0), stop=(k == KT - 1))
                    lg = logits_sb[:, tt * 512:(tt + 1) * 512]
                    nc.vector.tensor_tensor(out=lg, in0=psum_l[:], in1=tie_sb[:],
                                            op=mybir.AluOpType.add)
                    # routing for this chunk's 32 groups
                    gsl = slice(tt * GC, (tt + 1) * GC)
                    nc.vector.tensor_reduce(
                        out=gmax[:, gsl],
                        in_=lg.rearrange("e (g t) -> e g t", t=NE),
                        axis=mybir.AxisListType.X, op=mybir.AluOpType.max)
                    nc.vector.tensor_tensor(
                        out=iseq[:, tt * 512:(tt + 1) * 512].rearrange(
                            "e (g t) -> e g t", t=NE),
                        in0=lg.rearrange("e (g t) -> e g t", t=NE),
                        in1=gmax[:, gsl].unsqueeze(2).to_broadcast([NES, GC, NE]),
                        op=mybir.AluOpType.is_equal)
                    # tsel computed in place over iseq
                    nc.vector.tensor_tensor(
                        out=iseq[:, tt * 512:(tt + 1) * 512],
                        in0=iseq[:, tt * 512:(tt + 1) * 512],
                        in1=tokid_sb[:, tt * 512:(tt + 1) * 512],
                        op=mybir.AluOpType.mult)
                    nc.vector.tensor_reduce(
                        out=tid_f[:, gsl],
                        in_=iseq[:, tt * 512:(tt + 1) * 512].rearrange(
                            "e (g t) -> e g t", t=NE),
                        axis=mybir.AxisListType.X, op=mybir.AluOpType.max)

                # export routing table for the host-side combine
                tid_i32 = route.tile([NES, G], i32)
                nc.vector.tensor_copy(out=tid_i32[:], in_=tid_f[:])
                nc.scalar.dma_start(out=tids, in_=tid_i32[:])

                # gather idx tiles: transpose tid into [16, gh, es] psum layout
                psum_idx = psB.tile([16, 8, NES], f32, space="PSUM", tag="psidx")
                for gh in range(8):
                    nc.tensor.transpose(
                        out=psum_idx[:, gh, :],
                        in_=tid_f[:, gh * 16:(gh + 1) * 16],
                        identity=ident_sb[:])
                # idx_mega [128, 16 calls x 32 cols]; call h covers 4 es:
                # col j = e*8+gh, value = tid(es=4h+e, g=gh*16+p)
                idx_mega = idxp.tile([128, 16 * (GIDX // 16)], i16)
                for h in range(16):
                    nc.vector.tensor_copy(
                        out=idx_mega[0:16, h * 32:(h + 1) * 32].rearrange(
                            "p (e g) -> p e g", g=8),
                        in_=psum_idx[:, :, h * 4:(h + 1) * 4].transpose([0, 2, 1]))
                # replicate idx rows to all 128 partitions (Q7 channel reads)
                nc.sync.dma_start(out=idx_mega[16:32, :], in_=idx_mega[0:16, :])
                nc.sync.dma_start(out=idx_mega[32:64, :], in_=idx_mega[0:32, :])
                nc.sync.dma_start(out=idx_mega[64:128, :], in_=idx_mega[0:64, :])

            # ---- phase D: per es-group FFN, es processed in col/row-tiled
            # pairs ----
            with (
                tc.tile_pool(name="sp", bufs=3) as sp,
                tc.tile_pool(name="yp", bufs=2) as yp,
                tc.tile_pool(name="hp", bufs=3) as hp,
                tc.tile_pool(name="psH", bufs=2, space="PSUM") as psH,
                tc.tile_pool(name="psY", bufs=3, space="PSUM") as psY,
            ):
                for a in range(NCALLS):
                    f1_sb = wp.tile([128, KT, EG * ESZ], bfl, tag="f1")
                    nc.sync.dma_start(out=f1_sb[:], in_=f1s_r[:, a])
                    f2p_sb = wp.tile([128, EG // 2, DM], bfl, tag="f2")
                    nc.sync.dma_start(
                        out=f2p_sb[:],
                        in_=f2p[:, a * (EG // 2):(a + 1) * (EG // 2), :])

                    selT_halves = []
                    for half in range(2):
                        selTh = sp.tile([128, KT, GIDX], bfl, tag=f"selT{half}",
                                        name=f"selT_{a}_{half}")
                        nc.gpsimd.dma_gather(
                            out_ap=selTh[:],
                            in_ap=xbf_sb[:],
                            idxs_ap=idx_mega[:, (2 * a + half) * 32:
                                             (2 * a + half + 1) * 32],
                            num_idxs=GIDX, num_idxs_reg=GIDX, elem_size=DM,
                            transpose=True,
                            sbuf_tokens_per_rank=128,
                            sbuf_free_dim_per_rank=2 * DM,
                            sbuf_free_dim_pad_per_rank=0,
                            sbuf_byte_offset=0)
                        selT_halves.append(selTh)

                    y_sb = yp.tile([128, EG, DM], bfl)
                    for pl in range(EG // 2):          # es pair (8a+2pl, +1)
                        s0, s1 = 2 * pl, 2 * pl + 1    # slots within a
                        psum_h = psH.tile([128, G], f32, space="PSUM")
                        # up-proj: col-tiled pair, es0 -> array cols 0-63,
                        # es1 -> cols 64-127
                        for k in range(KT):
                            st = (k == 0)
                            sp_ = (k == KT - 1)
                            sel0 = selT_halves[s0 // 4]
                            sel1 = selT_halves[s1 // 4]
                            nc.tensor.matmul(
                                psum_h[0:64, :],
                                f1_sb[:, k, s0 * ESZ:(s0 + 1) * ESZ],
                                sel0[:, k, (s0 % 4) * G:(s0 % 4 + 1) * G],
                                start=st, stop=sp_, tile_position=(0, 0))
                            nc.tensor.matmul(
                                psum_h[64:128, :],
                                f1_sb[:, k, s1 * ESZ:(s1 + 1) * ESZ],
                                sel1[:, k, (s1 % 4) * G:(s1 % 4 + 1) * G],
                                start=st, stop=sp_, tile_position=(0, 64))
                        h_pair = hp.tile([128, G], bfl)
                        nc.scalar.activation(
                            out=h_pair[:], in_=psum_h[:],
                            func=mybir.ActivationFunctionType.Relu,
                            bias=biasP_sb[:, a * 4 + pl:a * 4 + pl + 1],
                            scale=1.0)
                        # down-proj: row-tiled pair, f of es0 on array rows
                        # 0-63, es1 on rows 64-127; disjoint PSUM outputs.
                        for n in range(2):
                            psum_t = psY.tile([128, 2, 512], f32, space="PSUM")
                            nc.tensor.matmul(
                                psum_t[:, 0, :],
                                h_pair[0:64, :],
                                f2p_sb[0:64, pl, n * 512:(n + 1) * 512],
                                start=True, stop=True)
                            nc.tensor.matmul(
                                psum_t[:, 1, :],
                                h_pair[64:128, :],
                                f2p_sb[64:128, pl, n * 512:(n + 1) * 512],
                                start=True, stop=True)
                            if n == 0:
                                nc.vector.tensor_copy(
                                    out=y_sb[:, s0:s1 + 1, 0:512],
                                    in_=psum_t[:])
                            else:
                                nc.scalar.copy(
                                    out=y_sb[:, s0:s1 + 1, 512:1024],
                                    in_=psum_t[:])

                    # dense write: [g, a, e, d] contiguous per partition
                    nc.scalar.dma_start(out=stage3_r[:, a], in_=y_sb[:])

    nc.compile()
    return nc


def _host_prep(x, controller, f1, f2, bias):
    """Returns (shared_map, per_core_maps)."""
    x = np.asarray(x, dtype=np.float32)
    c2c = np.ascontiguousarray(
        np.asarray(controller, np.float32).reshape(KT, 128, NES)
        .transpose(1, 0, 2).reshape(128, KT * NES))
    f1s = np.ascontiguousarray(
        np.asarray(f1, np.float32).reshape(DM, NES * ESZ)
        .reshape(KT, 128, NCALLS, 512).transpose(1, 2, 0, 3)
        .reshape(128, NCALLS * KT * 512)).astype(bf16)
    f2_ = np.asarray(f2, np.float32).reshape(NES, ESZ, DM)
    # pair layout: partition p = e01*64 + f, col = pair index
    f2p = np.ascontiguousarray(
        f2_.reshape(NES // 2, 2, ESZ, DM).transpose(1, 2, 0, 3)
        .reshape(128, NES // 2, DM)).astype(bf16)
    biasP = np.ascontiguousarray(
        np.asarray(bias, np.float32).reshape(NES // 2, 2, ESZ)
        .transpose(1, 2, 0).reshape(128, NES // 2))
    tie = np.linspace(0.0, 1e-6, NE, dtype=np.float32)
    tie512 = np.broadcast_to(np.tile(tie, 512 // NE), (NES, 512)).copy()
    tokid = np.broadcast_to(np.arange(SEQ, dtype=np.float32), (NES, SEQ)).copy()
    ident = np.eye(NES, dtype=np.float32)
    shared = dict(c2c=c2c, f1s=f1s, f2p=f2p, biasP=biasP, tie512=tie512,
                  tokid=tokid, ident=ident)
    per_core = []
    for b in range(NCORES):
        xb = x[b]
        xT = xb.T  # [DM, SEQ]
        xTc = np.ascontiguousarray(
            xT.reshape(KT, 128, TC, 512).transpose(1, 2, 0, 3)
            .reshape(128, TC * KT * 512))
        per_core.append(dict(
            xTc=xTc,
            xbf=np.ascontiguousarray(
                xb.astype(bf16).reshape(16, 128, DM).swapaxes(0, 1)
                .reshape(128, 16 * DM)),
        ))
    return shared, per_core


def _run(inputs, trace=False, tmpdir=None, trace_cores=None):
    if "nc" not in _CACHE:
        _CACHE["nc"] = _build_program()
    nc = _CACHE["nc"]
    shared, per_core = _host_prep(
        inputs["x"], inputs["controller"], inputs["f1"], inputs["f2"],
        inputs["bias"])
    in_maps = [dict(shared, **pc) for pc in per_core]
    res = run_bass_kernel_spmd(
        nc, in_maps, core_ids=list(range(NCORES)), trace=trace, tmpdir=tmpdir,
        trace_cores=trace_cores)
    out = np.zeros((NCORES, SEQ, DM), dtype=np.float32)
    for b in range(NCORES):
        # stage3 [g, a, e, d] -> (a, e, g) = es-major contribution rows
        st = np.asarray(res.results[b]["stage3"]).astype(np.float32)
        st = st.reshape(128, NCALLS, EG, DM).transpose(1, 2, 0, 3)
        tid = np.asarray(res.results[b]["tids"]).reshape(-1)  # [es*G] token ids
        np.add.at(out[b], tid, st.reshape(NES * G, DM))
    return out, res


def kernel(**inputs) -> np.ndarray:
    out, _ = _run(inputs)
    return out


# revision 27
# speedup vs baseline: 1.1393x; 1.0186x over previous
"""Trainium2 Bass kernel for nn_BatchSplitFF (expert-choice MoE routing + FFN).

Strategy (data-parallel over batch, 1 batch per NeuronCore, 8 cores):
  - fp32 logits on PE in [es, tok] orientation (routing must match the fp32
    reference argmax; bf16/fp32r logits flip routing decisions), N=512
    moving chunks; routing reductions run per-chunk, overlapped with the
    next chunk's matmuls.
  - dispatch: SWDGE dma_gather with SBUF source (sbuf_tokens_per_rank mode)
    reads selected token rows from the SBUF-resident bf16 copy of x --
    no HBM traffic for the 16.8MB of gathered rows.
  - expert FFN in bf16 on PE, es-pairs processed together:
      up-proj: col-tiled pair (es0 -> array cols 0-63, es1 -> cols 64-127)
      sharing one PSUM [128,128] tile; single ACT relu+bias per pair.
      down-proj: row-tiled pair (f of es0 on array rows 0-63, es1 on rows
      64-127) with f2 moving, disjoint PSUM outputs.
  - all DRAM tensors are laid out host-side so every DMA is contiguous
    per partition (16KB+ descriptors; strided descriptors measured 3-10x
    slower).
  - un-permute: y rows written densely to DRAM (bf16) as [g, a, e, d],
    routing table (tids) exported; host does the final scatter-add combine
    in fp32 (8.4M adds = 0.04% of FLOPs).
"""

import numpy as np
import ml_dtypes

import concourse.bass as bass
import concourse.mybir as mybir
import concourse.tile as tile
from concourse import bacc
from concourse.bass_utils import run_bass_kernel_spmd

bf16 = ml_dtypes.bfloat16
f32 = mybir.dt.float32
bfl = mybir.dt.bfloat16
i16 = mybir.dt.int16
i32 = mybir.dt.int32

DM, NE, ES, ESZ = 1024, 16, 4, 64
NES = NE * ES            # 64 (e,s) expert pairs
SEQ = 2048
G = SEQ // NE            # 128 groups per core
KT = DM // 128           # 8 contraction tiles
NCORES = 8
EG = 8                   # es-pairs per group-iteration
NCALLS = NES // EG       # 8 group-iterations
GIDX = 512               # idxs per dma_gather call (HW-validated limit)
TC = 4                   # token chunks for logits (512 tokens each)
GC = G // TC             # groups per token chunk (32)

_CACHE = {}


def _build_program():
    nc = bacc.Bacc("TRN2", target_bir_lowering=False, debug=False,
                   enable_asserts=False, num_devices=1)

    # [p, tt, k, 512]: xT chunk tt contiguous 16KB per partition
    xTc = nc.dram_tensor("xTc", [128, TC * KT * 512], f32,
                         kind="ExternalInput").ap()
    # token t at partition t%128, free stripe t//128 (gather source)
    xbf = nc.dram_tensor("xbf", [128, 16 * DM], bfl, kind="ExternalInput").ap()
    # [p, k, 64]: controller, contiguous per partition
    c2c = nc.dram_tensor("c2c", [128, KT * NES], f32, kind="ExternalInput").ap()
    # [p, a, k, 512]: f1 slab a contiguous 8KB per partition
    f1s = nc.dram_tensor("f1s", [128, NCALLS * KT * 512], bfl,
                         kind="ExternalInput").ap()
    # [e01*64+f, pair, d]: f2 pair layout (slab of 4 pairs contiguous)
    f2p = nc.dram_tensor("f2p", [128, NES // 2, DM], bfl,
                         kind="ExternalInput").ap()
    biasP = nc.dram_tensor("biasP", [128, NES // 2], f32,
                           kind="ExternalInput").ap()
    tie512 = nc.dram_tensor("tie512", [NES, 512], f32, kind="ExternalInput").ap()
    tokid = nc.dram_tensor("tokid", [NES, SEQ], f32, kind="ExternalInput").ap()
    ident = nc.dram_tensor("ident", [NES, NES], f32, kind="ExternalInput").ap()
    # [g, a, e, d]: y written contiguously 4KB per partition per pair
    stage3 = nc.dram_tensor("stage3", [128, NCALLS * EG * DM], bfl,
                            kind="ExternalOutput").ap()
    tids = nc.dram_tensor("tids", [NES, G], i32, kind="ExternalOutput").ap()

    with tile.TileContext(nc) as tc:
        with (
            tc.tile_pool(name="consts", bufs=1) as consts,
            tc.tile_pool(name="route", bufs=1) as route,
            tc.tile_pool(name="idxp", bufs=1) as idxp,
            tc.tile_pool(name="wp", bufs=2) as wp,
        ):
            # ---- constants into SBUF ----
            c_sb = consts.tile([128, KT, NES], f32)
            nc.sync.dma_start(out=c_sb[:],
                              in_=c2c.rearrange("p (k e) -> p k e", k=KT))
            biasP_sb = consts.tile([128, NES // 2], f32)
            nc.sync.dma_start(out=biasP_sb[:], in_=biasP)
            tie_sb = consts.tile([NES, 512], f32)
            nc.sync.dma_start(out=tie_sb[:], in_=tie512)
            tokid_sb = consts.tile([NES, SEQ], f32)
            nc.sync.dma_start(out=tokid_sb[:], in_=tokid)
            ident_sb = consts.tile([NES, NES], f32)
            nc.sync.dma_start(out=ident_sb[:], in_=ident)

            logits_sb = route.tile([NES, SEQ], f32)
            xbf_sb = consts.tile([128, 16 * DM], bfl)

            gmax = route.tile([NES, G], f32)
            iseq = route.tile([NES, SEQ], f32)
            tid_f = route.tile([NES, G], f32)

            xTc_r = xTc.rearrange("p (c k t) -> p c k t", c=TC, k=KT)
            f1s_r = f1s.rearrange("p (a k t) -> p a k t", a=NCALLS, k=KT)
            stage3_r = stage3.rearrange("p (a e d) -> p a e d", a=NCALLS, e=EG)

            # ---- phase B: fp32 logits + per-chunk routing ----
            with (
                tc.tile_pool(name="xtp", bufs=2) as xtp,
                tc.tile_pool(name="psB", bufs=2, space="PSUM") as psB,
            ):
                for tt in range(TC):
                    xt_t = xtp.tile([128, KT, 512], f32)
                    nc.sync.dma_start(out=xt_t[:], in_=xTc_r[:, tt])
                    if tt == TC - 1:
                        # gather source; queued behind the xT stream so the
                        # logits pipeline is never starved (needed at ~55us)
                        nc.sync.dma_start(out=xbf_sb[:], in_=xbf)
                    psum_l = psB.tile([NES, 512], f32, space="PSUM")
                    for k in range(KT):
                        nc.tensor.matmul(psum_l[:], c_sb[:, k, :], xt_t[:, k, :],
                                         start=(k == 0), stop=(k == KT - 1))
                    lg = logits_sb[:, tt * 512:(tt + 1) * 512]
                    nc.vector.tensor_tensor(out=lg, in0=psum_l[:], in1=tie_sb[:],
                                            op=mybir.AluOpType.add)
                    # routing for this chunk's 32 groups
                    gsl = slice(tt * GC, (tt + 1) * GC)
                    nc.vector.tensor_reduce(
                        out=gmax[:, gsl],
                        in_=lg.rearrange("e (g t) -> e g t", t=NE),
                        axis=mybir.AxisListType.X, op=mybir.AluOpType.max)
                    nc.vector.tensor_tensor(
                        out=iseq[:, tt * 512:(tt + 1) * 512].rearrange(
                            "e (g t) -> e g t", t=NE),
                        in0=lg.rearrange("e (g t) -> e g t", t=NE),
                        in1=gmax[:, gsl].unsqueeze(2).to_broadcast([NES, GC, NE]),
                        op=mybir.AluOpType.is_equal)
                    # tsel computed in place over iseq
                    nc.vector.tensor_tensor(
                        out=iseq[:, tt * 512:(tt + 1) * 512],
                        in0=iseq[:, tt * 512:(tt + 1) * 512],
                        in1=tokid_sb[:, tt * 512:(tt + 1) * 512],
                        op=mybir.AluOpType.mult)
                    nc.vector.tensor_reduce(
                        out=tid_f[:, gsl],
                        in_=iseq[:, tt * 512:(tt + 1) * 512].rearrange(
                            "e (g t) -> e g t", t=NE),
                        axis=mybir.AxisListType.X, op=mybir.AluOpType.max)

                # export routing table for the host-side combine
                tid_i32 = route.tile([NES, G], i32)
                nc.vector.tensor_copy(out=tid_i32[:], in_=tid_f[:])
                nc.scalar.dma_start(out=tids, in_=tid_i32[:])

                # gather idx tiles: transpose tid into [16, gh, es] psum layout
                psum_idx = psB.tile([16, 8, NES], f32, space="PSUM", tag="psidx")
                for gh in range(8):
                    nc.tensor.transpose(
                        out=psum_idx[:, gh, :],
                        in_=tid_f[:, gh * 16:(gh + 1) * 16],
                        identity=ident_sb[:])
                # idx_mega [128, 16 calls x 32 cols]; call h covers 4 es:
                # col j = e*8+gh, value = tid(es=4h+e, g=gh*16+p)
                idx_mega = idxp.tile([128, 16 * (GIDX // 16)], i16)
                for h in range(16):
                    nc.vector.tensor_copy(
                        out=idx_mega[0:16, h * 32:(h + 1) * 32].rearrange(
                            "p (e g) -> p e g", g=8),
                        in_=psum_idx[:, :, h * 4:(h + 1) * 4].transpose([0, 2, 1]))
                # replicate idx rows to all 128 partitions (Q7 channel reads)
                nc.sync.dma_start(out=idx_mega[16:32, :], in_=idx_mega[0:16, :])
                nc.sync.dma_start(out=idx_mega[32:64, :], in_=idx_mega[0:32, :])
                nc.sync.dma_start(out=idx_mega[64:128, :], in_=idx_mega[0:64, :])

            # ---- phase D: per es-group FFN, es processed in col/row-tiled
            # pairs ----
            with (
                tc.tile_pool(name="sp", bufs=3) as sp,
                tc.tile_pool(name="yp", bufs=2) as yp,
                tc.tile_pool(name="hp", bufs=3) as hp,
                tc.tile_pool(name="psH", bufs=2, space="PSUM") as psH,
                tc.tile_pool(name="psY", bufs=3, space="PSUM") as psY,
            ):
                for a in range(NCALLS):
                    f1_sb = wp.tile([128, KT, EG * ESZ], bfl, tag="f1")
                    nc.sync.dma_start(out=f1_sb[:], in_=f1s_r[:, a])
                    f2p_sb = wp.tile([128, EG // 2, DM], bfl, tag="f2")
                    nc.sync.dma_start(
                        out=f2p_sb[:],
                        in_=f2p[:, a * (EG // 2):(a + 1) * (EG // 2), :])

                    selT_halves = []
                    for half in range(2):
                        selTh = sp.tile([128, KT, GIDX], bfl, tag=f"selT{half}",
                                        name=f"selT_{a}_{half}")
                        nc.gpsimd.dma_gather(
                            out_ap=selTh[:],
                            in_ap=xbf_sb[:],
                            idxs_ap=idx_mega[:, (2 * a + half) * 32:
                                             (2 * a + half + 1) * 32],
                            num_idxs=GIDX, num_idxs_reg=GIDX, elem_size=DM,
                            transpose=True,
                            sbuf_tokens_per_rank=128,
                            sbuf_free_dim_per_rank=2 * DM,
                            sbuf_free_dim_pad_per_rank=0,
                            sbuf_byte_offset=0)
                        selT_halves.append(selTh)

                    y_sb = yp.tile([128, EG, DM], bfl)
                    for pl in range(EG // 2):          # es pair (8a+2pl, +1)
                        s0, s1 = 2 * pl, 2 * pl + 1    # slots within a
                        psum_h = psH.tile([128, G], f32, space="PSUM")
                        # up-proj: col-tiled pair, es0 -> array cols 0-63,
                        # es1 -> cols 64-127
                        for k in range(KT):
                            st = (k == 0)
                            sp_ = (k == KT - 1)
                            sel0 = selT_halves[s0 // 4]
                            sel1 = selT_halves[s1 // 4]
                            nc.tensor.matmul(
                                psum_h[0:64, :],
                                f1_sb[:, k, s0 * ESZ:(s0 + 1) * ESZ],
                                sel0[:, k, (s0 % 4) * G:(s0 % 4 + 1) * G],
                                start=st, stop=sp_, tile_position=(0, 0))
                            nc.tensor.matmul(
                                psum_h[64:128, :],
                                f1_sb[:, k, s1 * ESZ:(s1 + 1) * ESZ],
                                sel1[:, k, (s1 % 4) * G:(s1 % 4 + 1) * G],
                                start=st, stop=sp_, tile_position=(0, 64))
                        h_pair = hp.tile([128, G], bfl)
                        nc.scalar.activation(
                            out=h_pair[:], in_=psum_h[:],
                            func=mybir.ActivationFunctionType.Relu,
                            bias=biasP_sb[:, a * 4 + pl:a * 4 + pl + 1],
                            scale=1.0)
                        # down-proj: row-tiled pair, f of es0 on array rows
                        # 0-63, es1 on rows 64-127; disjoint PSUM outputs.
                        for n in range(2):
                            psum_t = psY.tile([128, 2, 512], f32, space="PSUM")
                            nc.tensor.matmul(
                                psum_t[:, 0, :],
                                h_pair[0:64, :],
                                f2p_sb[0:64, pl, n * 512:(n + 1) * 512],
                                start=True, stop=True)
                            nc.tensor.matmul(
                                psum_t[:, 1, :],
                                h_pair[64:128, :],
                                f2p_sb[64:128, pl, n * 512:(n + 1) * 512],
                                start=True, stop=True)
                            if n == 0:
                                nc.vector.tensor_copy(
                                    out=y_sb[:, s0:s1 + 1, 0:512],
                                    in_=psum_t[:])
                            else:
                                nc.scalar.copy(
                                    out=y_sb[:, s0:s1 + 1, 512:1024],
                                    in_=psum_t[:])

                    # dense write: [g, a, e, d] contiguous per partition;
                    # two halves to halve per-engine packet chains (less
                    # head-of-line blocking vs gather/load packets)
                    nc.scalar.dma_start(out=stage3_r[:, a, 0:4], in_=y_sb[:, 0:4])
                    nc.scalar.dma_start(out=stage3_r[:, a, 4:8], in_=y_sb[:, 4:8])

    nc.compile()
    return nc


def _host_prep(x, controller, f1, f2, bias):
    """Returns (shared_map, per_core_maps)."""
    x = np.asarray(x, dtype=np.float32)
    c2c = np.ascontiguousarray(
        np.asarray(controller, np.float32).reshape(KT, 128, NES)
        .transpose(1, 0, 2).reshape(128, KT * NES))
    f1s = np.ascontiguousarray(
        np.asarray(f1, np.float32).reshape(DM, NES * ESZ)
        .reshape(KT, 128, NCALLS, 512).transpose(1, 2, 0, 3)
        .reshape(128, NCALLS * KT * 512)).astype(bf16)
    f2_ = np.asarray(f2, np.float32).reshape(NES, ESZ, DM)
    # pair layout: partition p = e01*64 + f, col = pair index
    f2p = np.ascontiguousarray(
        f2_.reshape(NES // 2, 2, ESZ, DM).transpose(1, 2, 0, 3)
        .reshape(128, NES // 2, DM)).astype(bf16)
    biasP = np.ascontiguousarray(
        np.asarray(bias, np.float32).reshape(NES // 2, 2, ESZ)
        .transpose(1, 2, 0).reshape(128, NES // 2))
    tie = np.linspace(0.0, 1e-6, NE, dtype=np.float32)
    tie512 = np.broadcast_to(np.tile(tie, 512 // NE), (NES, 512)).copy()
    tokid = np.broadcast_to(np.arange(SEQ, dtype=np.float32), (NES, SEQ)).copy()
    ident = np.eye(NES, dtype=np.float32)
    shared = dict(c2c=c2c, f1s=f1s, f2p=f2p, biasP=biasP, tie512=tie512,
                  tokid=tokid, ident=ident)
    per_core = []
    for b in range(NCORES):
        xb = x[b]
        xT = xb.T  # [DM, SEQ]
        xTc = np.ascontiguousarray(
            xT.reshape(KT, 128, TC, 512).transpose(1, 2, 0, 3)
            .reshape(128, TC * KT * 512))
        per_core.append(dict(
            xTc=xTc,
            xbf=np.ascontiguousarray(
                xb.astype(bf16).reshape(16, 128, DM).swapaxes(0, 1)
                .reshape(128, 16 * DM)),
        ))
    return shared, per_core


def _run(inputs, trace=False, tmpdir=None, trace_cores=None):
    if "nc" not in _CACHE:
        _CACHE["nc"] = _build_program()
    nc = _CACHE["nc"]
    shared, per_core = _host_prep(
        inputs["x"], inputs["controller"], inputs["f1"], inputs["f2"],
        inputs["bias"])
    in_maps = [dict(shared, **pc) for pc in per_core]
    res = run_bass_kernel_spmd(
        nc, in_maps, core_ids=list(range(NCORES)), trace=trace, tmpdir=tmpdir,
        trace_cores=trace_cores)
    out = np.zeros((NCORES, SEQ, DM), dtype=np.float32)
    for b in range(NCORES):
        # stage3 [g, a, e, d] -> (a, e, g) = es-major contribution rows
        st = np.asarray(res.results[b]["stage3"]).astype(np.float32)
        st = st.reshape(128, NCALLS, EG, DM).transpose(1, 2, 0, 3)
        tid = np.asarray(res.results[b]["tids"]).reshape(-1)  # [es*G] token ids
        np.add.at(out[b], tid, st.reshape(NES * G, DM))
    return out, res


def kernel(**inputs) -> np.ndarray:
    out, _ = _run(inputs)
    return out
